# revision 8
# baseline (speedup 1.0000x reference)
"""Trainium2 Bass kernel for a token-embedding LSTM:
    x = emb[tokens]                               [B, T, E]
    LSTM over T steps (units=512), final h_T
    out = sigmoid(h_T @ W + b)                    [B, 1]

Sharding: data-parallel over batch. B=256 split as 32 rows per core
across 8 NeuronCores; weights replicated; no collectives.

Per-core dataflow:
  Phase B: gather embedding rows (time-major order), PE-transpose them,
    and precompute xproj = x @ [Wf|Wi|Wo|Wc] + b for all T*32 rows into
    DRAM (bf16).
  Phase C: sequential LSTM. Gates are stacked along PSUM partitions
    (f rows 0-31, i 32-63, o 64-95, c 96-127) in ONE [128, 512] psum
    bank; the four gate matmul chains run CONCURRENTLY in the four
    32-column groups of the PE array via tile_position col-tiling, so
    the 16 U-chunk matmuls cost ~4x512 streamed columns instead of 16x.
    Activations are consolidated (one sigmoid over 96 partitions + one
    tanh over 32) since engine cost is free-size-proportional. The
    elementwise c/h chain is split across DVE and GpSimd.
"""

import os
import sys

import numpy as np
import ml_dtypes

sys.path.insert(0, "/opt/trn_rl_repo")

import concourse.bacc as bacc
import concourse.bass as bass
import concourse.mybir as mybir
import concourse.tile as tile
from concourse.bass_utils import run_bass_kernel_spmd

F32 = mybir.dt.float32
BF16 = mybir.dt.bfloat16
I32 = mybir.dt.int32
AF = mybir.ActivationFunctionType

N_CORES = 8
B = 256
B_LOC = B // N_CORES  # 32
T_FULL = 512
EMB = 256
UNITS = 512
G = 4 * UNITS  # 2048 concatenated gate width, order [f | i | o | c]
VOCAB = 50000


def build_nc(T=T_FULL, unroll=32, num_devices=N_CORES):
    """Build the per-core Bass program. Same program runs on all cores."""
    rows = T * B_LOC
    n_mtiles = rows // 128
    assert rows % 128 == 0
    assert T % unroll == 0 and unroll % 4 == 0
    u2 = unroll // 4

    nc = bacc.Bacc(
        "TRN2", target_bir_lowering=False, debug=False, num_devices=num_devices
    )

    tokens_pm = nc.dram_tensor(
        "tokens_pm", [128, n_mtiles], I32, kind="ExternalInput"
    ).ap()
    emb_d = nc.dram_tensor("emb", [VOCAB, EMB], BF16, kind="ExternalInput").ap()
    wcat_d = nc.dram_tensor("wcat", [EMB, G], BF16, kind="ExternalInput").ap()
    ucat_d = nc.dram_tensor("ucat", [UNITS, G], BF16, kind="ExternalInput").ap()
    ones_d = nc.dram_tensor("ones", [1, 128], BF16, kind="ExternalInput").ap()
    bb_d = nc.dram_tensor("bb", [128, 1024], F32, kind="ExternalInput").ap()
    brow_d = nc.dram_tensor("brow", [1, G], BF16, kind="ExternalInput").ap()
    ident_d = nc.dram_tensor("ident", [128, 128], BF16, kind="ExternalInput").ap()
    wout_d = nc.dram_tensor("wout", [128, 4], BF16, kind="ExternalInput").ap()
    bout_d = nc.dram_tensor("bout", [B_LOC, 1], F32, kind="ExternalInput").ap()
    y_d = nc.dram_tensor("y", [B_LOC, 1], F32, kind="ExternalOutput").ap()

    with tile.TileContext(nc) as tc:
        with (
            tc.tile_pool(name="const", bufs=1) as constp,
            tc.tile_pool(name="dram", bufs=1, space="DRAM") as dramp,
        ):
            # resident constants
            u_sb = []
            for k in range(4):
                t = constp.tile([128, G], BF16, tag=f"u{k}")
                nc.sync.dma_start(t[:], ucat_d[k * 128 : (k + 1) * 128, :])
                u_sb.append(t)
            w_sb = []
            for c in range(2):
                t = constp.tile([128, G], BF16, tag=f"w{c}")
                nc.sync.dma_start(t[:], wcat_d[c * 128 : (c + 1) * 128, :])
                w_sb.append(t)
            ones_sb = constp.tile([1, 128], BF16, tag="ones")
            nc.sync.dma_start(ones_sb[:], ones_d[:])
            bb_sb = constp.tile([128, 1024], F32, tag="bb")
            nc.sync.dma_start(bb_sb[:], bb_d[:])
            brow_sb = constp.tile([1, G], BF16, tag="brow")
            nc.sync.dma_start(brow_sb[:], brow_d[:])
            id_sb = constp.tile([128, 128], BF16, tag="ident")
            nc.sync.dma_start(id_sb[:], ident_d[:])
            wout_sb = constp.tile([128, 4], BF16, tag="wout")
            nc.sync.dma_start(wout_sb[:], wout_d[:])
            bout_sb = constp.tile([B_LOC, 1], F32, tag="bout")
            nc.sync.dma_start(bout_sb[:], bout_d[:])
            tok_sb = constp.tile([128, n_mtiles], I32, tag="tok")
            nc.sync.dma_start(tok_sb[:], tokens_pm[:])

            xproj = dramp.tile([rows, G], BF16)

            # ---- Phase B: gather + transpose + xproj precompute ----
            with (
                tc.tile_pool(name="gat", bufs=3) as gatp,
                tc.tile_pool(name="xtp", bufs=3) as xtp,
                tc.tile_pool(name="xpo", bufs=3) as xpop,
                tc.tile_pool(name="psB", bufs=2, space="PSUM") as psB,
                tc.tile_pool(name="psX", bufs=1, space="PSUM") as psX,
            ):
                for m in range(n_mtiles):
                    xg = gatp.tile([128, EMB], BF16, tag="xg")
                    nc.gpsimd.indirect_dma_start(
                        out=xg[:],
                        out_offset=None,
                        in_=emb_d[:],
                        in_offset=bass.IndirectOffsetOnAxis(
                            ap=tok_sb[:, m : m + 1], axis=0
                        ),
                    )
                    xts = []
                    for c in range(2):
                        trp = psB.tile([128, 128], BF16, tag="trp")
                        nc.tensor.transpose(
                            trp[:], xg[:, c * 128 : (c + 1) * 128], id_sb[:]
                        )
                        xt = xtp.tile([128, 128], BF16, tag="xt")
                        nc.vector.tensor_copy(xt[:], trp[:])
                        xts.append(xt)
                    xpo = xpop.tile([128, G], BF16, tag="xpo")
                    for j in range(4):
                        nsl = slice(j * 512, (j + 1) * 512)
                        xps = psX.tile([128, 512], F32, tag=f"xps{j}")
                        first = True
                        if j >= 2:
                            nc.tensor.matmul(
                                xps[:],
                                lhsT=ones_sb[:, :],
                                rhs=brow_sb[:, nsl],
                                start=True,
                                stop=False,
                            )
                            first = False
                        for c in range(2):
                            nc.tensor.matmul(
                                xps[:],
                                lhsT=xts[c][:],
                                rhs=w_sb[c][:, nsl],
                                start=first,
                                stop=(c == 1),
                            )
                            first = False
                        if j < 2:
                            nc.vector.tensor_add(
                                xpo[:, nsl], xps[:], bb_sb[:, nsl]
                            )
                        else:
                            nc.scalar.copy(xpo[:, nsl], xps[:])
                    nc.gpsimd.dma_start(xproj[m * 128 : (m + 1) * 128, :], xpo[:])

            # ---- Phase C: recurrence ----
            with (
                tc.tile_pool(name="state", bufs=1) as statep,
                tc.tile_pool(name="xin", bufs=3) as xinp,
                tc.tile_pool(name="gsb", bufs=2) as gsbp,
                tc.tile_pool(name="tmp", bufs=2) as tmpp,
                tc.tile_pool(name="hsp", bufs=2) as hsp,
                tc.tile_pool(name="psG", bufs=2, space="PSUM") as psG,
                tc.tile_pool(name="psH", bufs=2, space="PSUM") as psH,
            ):
                hT_sb = statep.tile([128, 128], BF16, tag="hT")
                c_sb = statep.tile([B_LOC, UNITS], F32, tag="c")
                nc.vector.memset(hT_sb[:], 0.0)
                nc.vector.memset(c_sb[:], 0.0)

                def halfload(row0):
                    """DMA u2 steps of xproj ([u2*32, G] rows) into one
                    [32, u2*G] tile (batch in partitions, steps along free)."""
                    xq = xinp.tile([B_LOC, u2 * G], BF16, tag="xq")
                    src = xproj[row0, :].rearrange("(s b) g -> b s g", b=B_LOC)
                    nc.sync.dma_start(xq[:].rearrange("b (s g) -> b s g", s=u2), src)
                    return xq

                def inject(Gp, xq, s_local):
                    """Open the 4 per-gate accumulation groups with their
                    xproj slices, one per PE column group (concurrent).
                    Independent of the new hidden state, so these are
                    emitted during the previous step's tail."""
                    x0 = s_local * G
                    for g in range(4):
                        nc.tensor.matmul(
                            Gp[32 * g : 32 * g + 32, :],
                            lhsT=id_sb[0:B_LOC, 0:B_LOC],
                            rhs=xq[:, x0 + 512 * g : x0 + 512 * (g + 1)],
                            start=True,
                            stop=False,
                            skip_group_check=True,
                            tile_position=(0, 32 * g),
                        )

                def umms(Gp):
                    """16 recurrent matmuls, round-robin across the 4 col
                    groups so the 4 gate chains stream concurrently."""
                    for k in range(4):
                        for g in range(4):
                            nc.tensor.matmul(
                                Gp[32 * g : 32 * g + 32, :],
                                lhsT=hT_sb[:, 32 * k : 32 * k + 32],
                                rhs=u_sb[k][:, 512 * g : 512 * (g + 1)],
                                start=False,
                                stop=(k == 3),
                                skip_group_check=True,
                                tile_position=(0, 32 * g),
                            )

                def chain(Gp):
                    """Activations + c/h update for one step.

                    walrus requires tensor_tensor INPUTS to share a start
                    partition (outputs may cross), and partition accesses
                    must be 32/64/128-aligned. Gate order in PSUM is
                    [f|i|o|c]: sigmoid reads rows 0-95 aligned and writes
                    f@0 i@32 o@64; tanh writes chat@32 (paired with i);
                    thc lands @64 (paired with o)."""
                    gsb = gsbp.tile([96, 512], F32, tag="gsb")
                    cht = gsbp.tile([64, 512], F32, tag="cht")
                    nc.scalar.activation(gsb[0:96, :], Gp[0:96, :], AF.Sigmoid)
                    nc.scalar.activation(cht[32:64, :], Gp[96:128, :], AF.Tanh)
                    ct1 = tmpp.tile([B_LOC, UNITS], F32, tag="ct1")
                    ct2 = tmpp.tile([B_LOC, UNITS], F32, tag="ct2")
                    thc = tmpp.tile([96, UNITS], F32, tag="thc")
                    nc.gpsimd.tensor_mul(ct1[:], gsb[0:32, :], c_sb[:])
                    nc.vector.tensor_mul(ct2[:], gsb[32:64, :], cht[32:64, :])
                    nc.vector.tensor_add(c_sb[:], ct1[:], ct2[:])
                    nc.scalar.activation(thc[64:96, :], c_sb[:], AF.Tanh)
                    hs = hsp.tile([128, 128], BF16, tag="hs")
                    for cc in range(4):
                        eng = nc.vector if cc % 2 == 0 else nc.gpsimd
                        eng.tensor_mul(
                            hs[32 * cc : 32 * cc + 32, :],
                            gsb[64:96, 128 * cc : 128 * (cc + 1)],
                            thc[64:96, 128 * cc : 128 * (cc + 1)],
                        )
                    htp = psH.tile([128, 128], BF16, tag="htp")
                    nc.tensor.transpose(htp[:], hs[:], id_sb[:])
                    nc.vector.tensor_copy(hT_sb[:], htp[:])

                def iteration2(iv):
                    xqs = []
                    for q in range(4):
                        xqs.append(
                            halfload(bass.ds(iv + q * (u2 * B_LOC), u2 * B_LOC))
                        )
                    Gcur = psG.tile([128, 512], F32, tag="Ga")
                    inject(Gcur, xqs[0], 0)
                    for step_idx in range(unroll):
                        q, s = divmod(step_idx, u2)
                        umms(Gcur)
                        if step_idx + 1 < unroll:
                            q2, s2 = divmod(step_idx + 1, u2)
                            nxt = psG.tile(
                                [128, 512], F32, tag="Gb" if step_idx % 2 == 0 else "Ga"
                            )
                            inject(nxt, xqs[q2], s2)
                        else:
                            nxt = None
                        chain(Gcur)
                        Gcur = nxt

                n_iters = T // unroll
                if n_iters == 1:
                    iteration2(0)
                else:
                    with tc.For_i(
                        0,
                        rows,
                        B_LOC * unroll,
                        staggered_reset=True,
                        hint_engines=(
                            mybir.EngineType.PE,
                            mybir.EngineType.DVE,
                            mybir.EngineType.Activation,
                        ),
                    ) as iv:
                        iteration2(iv)

                # final projection + sigmoid
                yps = psH.tile([B_LOC, 1], F32, tag="htp")
                for k in range(4):
                    nc.tensor.matmul(
                        yps[:],
                        lhsT=hT_sb[:, k * 32 : (k + 1) * 32],
                        rhs=wout_sb[:, k : k + 1],
                        start=(k == 0),
                        stop=(k == 3),
                    )
                ysb = tmpp.tile([B_LOC, 1], F32, tag="ysb")
                nc.scalar.activation(ysb[:], yps[:], AF.Sigmoid, bias=bout_sb[:, 0:1])
                nc.sync.dma_start(y_d[:], ysb[:])

    nc.compile()
    return nc


def prep_inputs(tokens, emb, Wf, Uf, bf, Wi, Ui, bi, Wc, Uc, bc, Wo, Uo, bo, W, b):
    """Host-side prep: concat gate weights (order [c|f|i|o]), cast to
    bf16, shard tokens."""
    bf16 = ml_dtypes.bfloat16
    wcat = np.concatenate([Wf, Wi, Wo, Wc], axis=1).astype(bf16)  # [E, G]
    ucat = np.concatenate([Uf, Ui, Uo, Uc], axis=1).astype(bf16)  # [U, G]
    bcat = np.concatenate([bf, bi, bo, bc], axis=0).astype(np.float32)  # [G]
    brow = bcat[None, :].astype(bf16)
    bb = np.broadcast_to(bcat[None, :1024], (128, 1024)).copy()
    ones = np.ones((1, 128), bf16)
    emb_bf = np.asarray(emb, np.float32).astype(bf16)
    ident = np.eye(128, dtype=bf16)
    wout = np.ascontiguousarray(
        np.asarray(W, np.float32).reshape(4, 128).T
    ).astype(bf16)  # [128, 4]; wout[p, k] = W[k*128 + p]
    bout = np.full((B_LOC, 1), float(np.asarray(b).reshape(-1)[0]), np.float32)

    tokens = np.asarray(tokens)
    T = tokens.shape[1]
    n_mtiles = T * B_LOC // 128
    per_core = []
    for core in range(N_CORES):
        tok = tokens[core * B_LOC : (core + 1) * B_LOC]  # [B_LOC, T]
        tok_tm = np.ascontiguousarray(tok.T).reshape(-1)  # row = t*B_LOC + b
        tok_pm = np.ascontiguousarray(
            tok_tm.reshape(n_mtiles, 128).T
        ).astype(np.int32)  # [128, n_mtiles]
        per_core.append(
            dict(
                tokens_pm=tok_pm,
                emb=emb_bf,
                wcat=wcat,
                ucat=ucat,
                ones=ones,
                brow=brow,
                bb=bb,
                ident=ident,
                wout=wout,
                bout=bout,
            )
        )
    return per_core


_NC_CACHE = {}
LAST_RESULT = None


def kernel(**inputs):
    global LAST_RESULT
    key = "full"
    if key not in _NC_CACHE:
        _NC_CACHE[key] = build_nc()
    nc = _NC_CACHE[key]
    in_maps = prep_inputs(**inputs)
    res = run_bass_kernel_spmd(nc, in_maps, core_ids=list(range(N_CORES)))
    LAST_RESULT = res
    out = np.concatenate([r["y"] for r in res.results], axis=0)
    return out.astype(np.float32)


# revision 12
# speedup vs baseline: 1.4272x; 1.4272x over previous
"""Trainium2 Bass kernel for a token-embedding LSTM:
    x = emb[tokens]                               [B, T, E]
    LSTM over T steps (units=512), final h_T
    out = sigmoid(h_T @ W + b)                    [B, 1]

Sharding: data-parallel over batch. B=256 split as 32 rows per core
across 8 NeuronCores; weights replicated; no collectives.

Per-core dataflow:
  Phase B: gather embedding rows (time-major order), PE-transpose them,
    and precompute xproj = x @ [Wf|Wi|Wo|Wc] + b for all T*32 rows into
    DRAM (bf16).
  Phase C: sequential LSTM. Gate weight columns are host-permuted to
    unit-chunk-major order so one [128, 512] PSUM bank holds the
    repacked layout G[32*uc + b, 128*g + u]: partition = batch x
    unit-chunk, free = gate x unit-within-chunk. The four unit-chunk
    matmul chains run CONCURRENTLY in the four 32-column groups of the
    PE array via tile_position col-tiling (16 U matmuls cost ~4x512
    streamed columns), and every elementwise op is a [128, 128]
    full-partition tile, including h which lands directly in the
    strip-stacked layout the per-step PE transpose needs.
"""

import os
import sys

import numpy as np
import ml_dtypes

sys.path.insert(0, "/opt/trn_rl_repo")

import concourse.bacc as bacc
import concourse.bass as bass
import concourse.mybir as mybir
import concourse.tile as tile
from concourse.bass_utils import run_bass_kernel_spmd

F32 = mybir.dt.float32
BF16 = mybir.dt.bfloat16
I32 = mybir.dt.int32
AF = mybir.ActivationFunctionType

N_CORES = 8
B = 256
B_LOC = B // N_CORES  # 32
T_FULL = 512
EMB = 256
UNITS = 512
G = 4 * UNITS  # 2048 concatenated gate width, order [f | i | o | c]
VOCAB = 50000


def build_nc(T=T_FULL, unroll=32, num_devices=N_CORES):
    """Build the per-core Bass program. Same program runs on all cores."""
    rows = T * B_LOC
    n_mtiles = rows // 128
    assert rows % 128 == 0
    assert T % unroll == 0 and unroll % 4 == 0
    u2 = unroll // 4

    nc = bacc.Bacc(
        "TRN2", target_bir_lowering=False, debug=False, num_devices=num_devices
    )

    tokens_pm = nc.dram_tensor(
        "tokens_pm", [128, n_mtiles], I32, kind="ExternalInput"
    ).ap()
    emb_d = nc.dram_tensor("emb", [VOCAB, EMB], BF16, kind="ExternalInput").ap()
    wcat_d = nc.dram_tensor("wcat", [EMB, G], BF16, kind="ExternalInput").ap()
    ucat_d = nc.dram_tensor("ucat", [UNITS, G], BF16, kind="ExternalInput").ap()
    ones_d = nc.dram_tensor("ones", [1, 128], BF16, kind="ExternalInput").ap()
    bb_d = nc.dram_tensor("bb", [128, 1024], F32, kind="ExternalInput").ap()
    brow_d = nc.dram_tensor("brow", [1, G], BF16, kind="ExternalInput").ap()
    ident_d = nc.dram_tensor("ident", [128, 128], BF16, kind="ExternalInput").ap()
    wout_d = nc.dram_tensor("wout", [128, 4], BF16, kind="ExternalInput").ap()
    bout_d = nc.dram_tensor("bout", [B_LOC, 1], F32, kind="ExternalInput").ap()
    y_d = nc.dram_tensor("y", [B_LOC, 1], F32, kind="ExternalOutput").ap()

    with tile.TileContext(nc) as tc:
        with (
            tc.tile_pool(name="const", bufs=1) as constp,
            tc.tile_pool(name="dram", bufs=1, space="DRAM") as dramp,
        ):
            # resident constants
            u_sb = []
            for k in range(4):
                t = constp.tile([128, G], BF16, tag=f"u{k}")
                nc.sync.dma_start(t[:], ucat_d[k * 128 : (k + 1) * 128, :])
                u_sb.append(t)
            w_sb = []
            for c in range(2):
                t = constp.tile([128, G], BF16, tag=f"w{c}")
                nc.sync.dma_start(t[:], wcat_d[c * 128 : (c + 1) * 128, :])
                w_sb.append(t)
            ones_sb = constp.tile([1, 128], BF16, tag="ones")
            nc.sync.dma_start(ones_sb[:], ones_d[:])
            bb_sb = constp.tile([128, 1024], F32, tag="bb")
            nc.sync.dma_start(bb_sb[:], bb_d[:])
            brow_sb = constp.tile([1, G], BF16, tag="brow")
            nc.sync.dma_start(brow_sb[:], brow_d[:])
            id_sb = constp.tile([128, 128], BF16, tag="ident")
            nc.sync.dma_start(id_sb[:], ident_d[:])
            wout_sb = constp.tile([128, 4], BF16, tag="wout")
            nc.sync.dma_start(wout_sb[:], wout_d[:])
            bout_sb = constp.tile([B_LOC, 1], F32, tag="bout")
            nc.sync.dma_start(bout_sb[:], bout_d[:])
            tok_sb = constp.tile([128, n_mtiles], I32, tag="tok")
            nc.sync.dma_start(tok_sb[:], tokens_pm[:])

            xproj = dramp.tile([rows, G], BF16)

            # ---- Phase B: gather + transpose + xproj precompute ----
            with (
                tc.tile_pool(name="gat", bufs=3) as gatp,
                tc.tile_pool(name="xtp", bufs=3) as xtp,
                tc.tile_pool(name="xpo", bufs=3) as xpop,
                tc.tile_pool(name="psB", bufs=2, space="PSUM") as psB,
                tc.tile_pool(name="psX", bufs=1, space="PSUM") as psX,
            ):
                for m in range(n_mtiles):
                    xg = gatp.tile([128, EMB], BF16, tag="xg")
                    nc.gpsimd.indirect_dma_start(
                        out=xg[:],
                        out_offset=None,
                        in_=emb_d[:],
                        in_offset=bass.IndirectOffsetOnAxis(
                            ap=tok_sb[:, m : m + 1], axis=0
                        ),
                    )
                    xts = []
                    for c in range(2):
                        trp = psB.tile([128, 128], BF16, tag="trp")
                        nc.tensor.transpose(
                            trp[:], xg[:, c * 128 : (c + 1) * 128], id_sb[:]
                        )
                        xt = xtp.tile([128, 128], BF16, tag="xt")
                        nc.vector.tensor_copy(xt[:], trp[:])
                        xts.append(xt)
                    xpo = xpop.tile([128, G], BF16, tag="xpo")
                    for j in range(4):
                        nsl = slice(j * 512, (j + 1) * 512)
                        xps = psX.tile([128, 512], F32, tag=f"xps{j}")
                        first = True
                        if j >= 2:
                            nc.tensor.matmul(
                                xps[:],
                                lhsT=ones_sb[:, :],
                                rhs=brow_sb[:, nsl],
                                start=True,
                                stop=False,
                            )
                            first = False
                        for c in range(2):
                            nc.tensor.matmul(
                                xps[:],
                                lhsT=xts[c][:],
                                rhs=w_sb[c][:, nsl],
                                start=first,
                                stop=(c == 1),
                            )
                            first = False
                        if j < 2:
                            nc.vector.tensor_add(
                                xpo[:, nsl], xps[:], bb_sb[:, nsl]
                            )
                        else:
                            nc.scalar.copy(xpo[:, nsl], xps[:])
                    nc.gpsimd.dma_start(xproj[m * 128 : (m + 1) * 128, :], xpo[:])

            # ---- Phase C: recurrence ----
            with (
                tc.tile_pool(name="state", bufs=1) as statep,
                tc.tile_pool(name="xin", bufs=3) as xinp,
                tc.tile_pool(name="gsb", bufs=2) as gsbp,
                tc.tile_pool(name="tmp", bufs=2) as tmpp,
                tc.tile_pool(name="hsp", bufs=2) as hsp,
                tc.tile_pool(name="psG", bufs=2, space="PSUM") as psG,
                tc.tile_pool(name="psH", bufs=2, space="PSUM") as psH,
            ):
                hT_sb = statep.tile([128, 128], BF16, tag="hT")
                c_sb = statep.tile([128, 128], F32, tag="c")
                nc.vector.memset(hT_sb[:], 0.0)
                nc.vector.memset(c_sb[:], 0.0)

                def halfload(row0):
                    """DMA u2 steps of xproj ([u2*32, G] rows) into one
                    [32, u2*G] tile (batch in partitions, steps along free)."""
                    xq = xinp.tile([B_LOC, u2 * G], BF16, tag="xq")
                    src = xproj[row0, :].rearrange("(s b) g -> b s g", b=B_LOC)
                    nc.sync.dma_start(xq[:].rearrange("b (s g) -> b s g", s=u2), src)
                    return xq

                def inject(Gp, xq, s_local):
                    """Open the 4 per-unit-chunk accumulation groups with
                    their xproj slices, one per PE column group
                    (concurrent). Independent of the new hidden state, so
                    these are emitted during the previous step's tail."""
                    x0 = s_local * G
                    for uc in range(4):
                        nc.tensor.matmul(
                            Gp[32 * uc : 32 * uc + 32, :],
                            lhsT=id_sb[0:B_LOC, 0:B_LOC],
                            rhs=xq[:, x0 + 512 * uc : x0 + 512 * (uc + 1)],
                            start=True,
                            stop=False,
                            skip_group_check=True,
                            tile_position=(0, 32 * uc),
                        )

                def umms(Gp):
                    """16 recurrent matmuls, round-robin across the 4 col
                    groups so the 4 unit-chunk chains stream concurrently."""
                    for k in range(4):
                        for uc in range(4):
                            nc.tensor.matmul(
                                Gp[32 * uc : 32 * uc + 32, :],
                                lhsT=hT_sb[:, 32 * k : 32 * k + 32],
                                rhs=u_sb[k][:, 512 * uc : 512 * (uc + 1)],
                                start=False,
                                stop=(k == 3),
                                skip_group_check=True,
                                tile_position=(0, 32 * uc),
                            )

                def chain(Gp):
                    """Activations + c/h update for one step.

                    Gate columns are host-permuted so PSUM holds the
                    repacked layout G[32*uc + b, 128*g + u] = gate g of
                    batch b, unit 128*uc + u. Every elementwise tensor is
                    [128, 128] (full partitions, short free dim) at base
                    partition 0, and h lands directly in the strip-stacked
                    layout the PE transpose needs."""
                    gsb = gsbp.tile([128, 384], F32, tag="gsb")
                    cht = gsbp.tile([128, 128], F32, tag="cht")
                    nc.scalar.activation(cht[:], Gp[:, 384:512], AF.Tanh)
                    nc.scalar.activation(gsb[:], Gp[:, 0:384], AF.Sigmoid)
                    ct1 = tmpp.tile([128, 128], F32, tag="ct1")
                    ct2 = tmpp.tile([128, 128], F32, tag="ct2")
                    thc = tmpp.tile([128, 128], F32, tag="thc")
                    nc.vector.tensor_mul(ct1[:], gsb[:, 0:128], c_sb[:])
                    nc.vector.tensor_mul(ct2[:], gsb[:, 128:256], cht[:])
                    nc.vector.tensor_add(c_sb[:], ct1[:], ct2[:])
                    nc.scalar.activation(thc[:], c_sb[:], AF.Tanh)
                    hs = hsp.tile([128, 128], BF16, tag="hs")
                    nc.vector.tensor_mul(hs[:], gsb[:, 256:384], thc[:])
                    htp = psH.tile([128, 128], BF16, tag="htp")
                    nc.tensor.transpose(htp[:], hs[:], id_sb[:])
                    nc.vector.tensor_copy(hT_sb[:], htp[:])

                def iteration2(iv):
                    xqs = []
                    for q in range(4):
                        xqs.append(
                            halfload(bass.ds(iv + q * (u2 * B_LOC), u2 * B_LOC))
                        )
                    Gcur = psG.tile([128, 512], F32, tag="Ga")
                    inject(Gcur, xqs[0], 0)
                    for step_idx in range(unroll):
                        q, s = divmod(step_idx, u2)
                        umms(Gcur)
                        if step_idx + 1 < unroll:
                            q2, s2 = divmod(step_idx + 1, u2)
                            nxt = psG.tile(
                                [128, 512], F32, tag="Gb" if step_idx % 2 == 0 else "Ga"
                            )
                            inject(nxt, xqs[q2], s2)
                        else:
                            nxt = None
                        chain(Gcur)
                        Gcur = nxt

                n_iters = T // unroll
                if n_iters == 1:
                    iteration2(0)
                else:
                    with tc.For_i(
                        0,
                        rows,
                        B_LOC * unroll,
                        staggered_reset=True,
                        hint_engines=(
                            mybir.EngineType.PE,
                            mybir.EngineType.DVE,
                            mybir.EngineType.Activation,
                        ),
                    ) as iv:
                        iteration2(iv)

                # final projection + sigmoid
                yps = psH.tile([B_LOC, 1], F32, tag="htp")
                for k in range(4):
                    nc.tensor.matmul(
                        yps[:],
                        lhsT=hT_sb[:, k * 32 : (k + 1) * 32],
                        rhs=wout_sb[:, k : k + 1],
                        start=(k == 0),
                        stop=(k == 3),
                    )
                ysb = tmpp.tile([B_LOC, 1], F32, tag="ysb")
                nc.scalar.activation(ysb[:], yps[:], AF.Sigmoid, bias=bout_sb[:, 0:1])
                nc.sync.dma_start(y_d[:], ysb[:])

    nc.compile()
    return nc


def prep_inputs(tokens, emb, Wf, Uf, bf, Wi, Ui, bi, Wc, Uc, bc, Wo, Uo, bo, W, b):
    """Host-side prep: concat gate weights (order [c|f|i|o]), cast to
    bf16, shard tokens."""
    bf16 = ml_dtypes.bfloat16
    # gate-major concat [f|i|o|c], then permute columns to the repacked
    # unit-chunk-major order: new[512*uc + 128*g + u] = old[512*g + 128*uc + u]
    perm = np.array(
        [
            512 * g + 128 * uc + u
            for uc in range(4)
            for g in range(4)
            for u in range(128)
        ]
    )
    wcat = np.concatenate([Wf, Wi, Wo, Wc], axis=1)[:, perm].astype(bf16)  # [E, G]
    ucat = np.concatenate([Uf, Ui, Uo, Uc], axis=1)[:, perm].astype(bf16)  # [U, G]
    bcat = np.concatenate([bf, bi, bo, bc], axis=0)[perm].astype(np.float32)  # [G]
    brow = bcat[None, :].astype(bf16)
    bb = np.broadcast_to(bcat[None, :1024], (128, 1024)).copy()
    ones = np.ones((1, 128), bf16)
    emb_bf = np.asarray(emb, np.float32).astype(bf16)
    ident = np.eye(128, dtype=bf16)
    wout = np.ascontiguousarray(
        np.asarray(W, np.float32).reshape(4, 128).T
    ).astype(bf16)  # [128, 4]; wout[p, k] = W[k*128 + p]
    bout = np.full((B_LOC, 1), float(np.asarray(b).reshape(-1)[0]), np.float32)

    tokens = np.asarray(tokens)
    T = tokens.shape[1]
    n_mtiles = T * B_LOC // 128
    per_core = []
    for core in range(N_CORES):
        tok = tokens[core * B_LOC : (core + 1) * B_LOC]  # [B_LOC, T]
        tok_tm = np.ascontiguousarray(tok.T).reshape(-1)  # row = t*B_LOC + b
        tok_pm = np.ascontiguousarray(
            tok_tm.reshape(n_mtiles, 128).T
        ).astype(np.int32)  # [128, n_mtiles]
        per_core.append(
            dict(
                tokens_pm=tok_pm,
                emb=emb_bf,
                wcat=wcat,
                ucat=ucat,
                ones=ones,
                brow=brow,
                bb=bb,
                ident=ident,
                wout=wout,
                bout=bout,
            )
        )
    return per_core


_NC_CACHE = {}
LAST_RESULT = None


def kernel(**inputs):
    global LAST_RESULT
    key = "full"
    if key not in _NC_CACHE:
        _NC_CACHE[key] = build_nc()
    nc = _NC_CACHE[key]
    in_maps = prep_inputs(**inputs)
    res = run_bass_kernel_spmd(nc, in_maps, core_ids=list(range(N_CORES)))
    LAST_RESULT = res
    out = np.concatenate([r["y"] for r in res.results], axis=0)
    return out.astype(np.float32)


# revision 18
# speedup vs baseline: 1.5544x; 1.0891x over previous
"""Trainium2 Bass kernel for a token-embedding LSTM:
    x = emb[tokens]                               [B, T, E]
    LSTM over T steps (units=512), final h_T
    out = sigmoid(h_T @ W + b)                    [B, 1]

Sharding: data-parallel over batch. B=256 split as 32 rows per core
across 8 NeuronCores; weights replicated; no collectives.

Per-core dataflow (single software-pipelined loop):
  xproj producer: gather embedding rows for one m-tile (128 rows =
    4 steps x 32 batch), PE-transpose, compute x @ [Wf|Wi|Wo|Wc] + b
    (bias via ones-row matmul) into an 8-slot SBUF ring tile. Producers
    for iteration i+1 run inside iteration i's chain bubbles, which
    also keeps the PE array warm (K=8/8 clock).
  Recurrence: gate weight columns are host-permuted to unit-chunk-major
    order so one [128, 512] PSUM bank holds the repacked layout
    G[32*uc + b, 128*g + u] (partition = batch x unit-chunk, free =
    gate x unit-within-chunk). The four unit-chunk matmul chains run
    CONCURRENTLY in the four 32-column groups of the PE array via
    tile_position col-tiling; the xproj inject reads the ring tile at
    row-group 32*(step%4). Every elementwise op is a [128, 128]
    full-partition tile, and h lands directly in the strip-stacked
    layout the per-step PE transpose needs.
"""

import os
import sys

import numpy as np
import ml_dtypes

sys.path.insert(0, "/opt/trn_rl_repo")

import concourse.bacc as bacc
import concourse.bass as bass
import concourse.mybir as mybir
import concourse.tile as tile
from concourse.bass_utils import run_bass_kernel_spmd

F32 = mybir.dt.float32
BF16 = mybir.dt.bfloat16
I32 = mybir.dt.int32
AF = mybir.ActivationFunctionType

N_CORES = 8
B = 256
B_LOC = B // N_CORES  # 32
T_FULL = 512
EMB = 256
UNITS = 512
G = 4 * UNITS  # 2048 gate width, unit-chunk-major permuted
VOCAB = 50000
RING = 8  # xproj ring tiles (= quads per 32-step iteration)


def build_nc(T=T_FULL, unroll=32, num_devices=N_CORES):
    """Build the per-core Bass program. Same program runs on all cores."""
    rows = T * B_LOC
    n_mtiles = rows // 128
    assert rows % 128 == 0
    assert T % unroll == 0 and unroll == 4 * RING

    nc = bacc.Bacc(
        "TRN2", target_bir_lowering=False, debug=False, num_devices=num_devices
    )

    tokens_pm = nc.dram_tensor(
        "tokens_pm", [128, n_mtiles + RING], I32, kind="ExternalInput"
    ).ap()
    emb_d = nc.dram_tensor("emb", [VOCAB, EMB], BF16, kind="ExternalInput").ap()
    wcat_d = nc.dram_tensor("wcat", [EMB, G], BF16, kind="ExternalInput").ap()
    ucat_d = nc.dram_tensor("ucat", [UNITS, G], BF16, kind="ExternalInput").ap()
    ones_d = nc.dram_tensor("ones", [1, 128], BF16, kind="ExternalInput").ap()
    brow_d = nc.dram_tensor("brow", [1, G], BF16, kind="ExternalInput").ap()
    ident_d = nc.dram_tensor("ident", [128, 128], BF16, kind="ExternalInput").ap()
    wout_d = nc.dram_tensor("wout", [128, 4], BF16, kind="ExternalInput").ap()
    bout_d = nc.dram_tensor("bout", [B_LOC, 1], F32, kind="ExternalInput").ap()
    y_d = nc.dram_tensor("y", [B_LOC, 1], F32, kind="ExternalOutput").ap()

    with tile.TileContext(nc) as tc:
        with tc.tile_pool(name="const", bufs=1) as constp:
            # resident constants
            u_sb = []
            for k in range(4):
                t = constp.tile([128, G], BF16, tag=f"u{k}")
                nc.sync.dma_start(t[:], ucat_d[k * 128 : (k + 1) * 128, :])
                u_sb.append(t)
            w_sb = []
            for c in range(2):
                t = constp.tile([128, G], BF16, tag=f"w{c}")
                nc.sync.dma_start(t[:], wcat_d[c * 128 : (c + 1) * 128, :])
                w_sb.append(t)
            ones_sb = constp.tile([1, 128], BF16, tag="ones")
            nc.sync.dma_start(ones_sb[:], ones_d[:])
            brow_sb = constp.tile([1, G], BF16, tag="brow")
            nc.sync.dma_start(brow_sb[:], brow_d[:])
            id_sb = constp.tile([128, 128], BF16, tag="ident")
            nc.sync.dma_start(id_sb[:], ident_d[:])
            wout_sb = constp.tile([128, 4], BF16, tag="wout")
            nc.sync.dma_start(wout_sb[:], wout_d[:])
            bout_sb = constp.tile([B_LOC, 1], F32, tag="bout")
            nc.sync.dma_start(bout_sb[:], bout_d[:])


            with (
                tc.tile_pool(name="state", bufs=1) as statep,
                tc.tile_pool(name="gat", bufs=3) as gatp,
                tc.tile_pool(name="xtp", bufs=3) as xtp,
                tc.tile_pool(name="xpo", bufs=RING) as xpop,
                tc.tile_pool(name="gsb", bufs=2) as gsbp,
                tc.tile_pool(name="tmp", bufs=2) as tmpp,
                tc.tile_pool(name="hsp", bufs=2) as hsp,
                tc.tile_pool(name="psG", bufs=3, space="PSUM") as psG,
                tc.tile_pool(name="psH", bufs=1, space="PSUM") as psH,
                tc.tile_pool(name="psB", bufs=1, space="PSUM") as psB,
                tc.tile_pool(name="psX", bufs=2, space="PSUM") as psX,
            ):
                hT_sb = statep.tile([128, 128], BF16, tag="hT")
                c_sb = statep.tile([128, 128], F32, tag="c")
                tok_stage = statep.tile([128, RING], I32, tag="tokstage")
                nc.vector.memset(hT_sb[:], 0.0)
                nc.vector.memset(c_sb[:], 0.0)

                def stage_tokens(col):
                    """DMA RING token columns [col, col+RING) from DRAM
                    into the staging tile (indirect-gather offsets must
                    be static SBUF APs, so the dynamic indexing happens
                    here on the DRAM side)."""
                    nc.sync.dma_start(
                        tok_stage[:], tokens_pm[:, bass.ds(col, RING)]
                    )

                def producer(xpo, j):
                    """Gather + transpose + xproj for the m-tile whose
                    tokens are in staging column j, into ring tile xpo."""
                    xg = gatp.tile([128, EMB], BF16, tag="xg")
                    nc.gpsimd.indirect_dma_start(
                        out=xg[:],
                        out_offset=None,
                        in_=emb_d[:],
                        in_offset=bass.IndirectOffsetOnAxis(
                            ap=tok_stage[:, j : j + 1], axis=0
                        ),
                    )
                    xts = []
                    for c in range(2):
                        trp = psB.tile([128, 128], BF16, tag="trp")
                        nc.tensor.transpose(
                            trp[:], xg[:, c * 128 : (c + 1) * 128], id_sb[:]
                        )
                        xt = xtp.tile([128, 128], BF16, tag="xt")
                        nc.vector.tensor_copy(xt[:], trp[:])
                        xts.append(xt)
                    for j4 in range(4):
                        nsl = slice(j4 * 512, (j4 + 1) * 512)
                        xps = psX.tile([128, 512], F32, tag="xps")
                        nc.tensor.matmul(
                            xps[:],
                            lhsT=ones_sb[:, :],
                            rhs=brow_sb[:, nsl],
                            start=True,
                            stop=False,
                        )
                        for c in range(2):
                            nc.tensor.matmul(
                                xps[:],
                                lhsT=xts[c][:],
                                rhs=w_sb[c][:, nsl],
                                start=False,
                                stop=(c == 1),
                            )
                        if j4 % 2 == 0:
                            nc.scalar.copy(xpo[:, nsl], xps[:])
                        else:
                            nc.vector.tensor_copy(xpo[:, nsl], xps[:])

                def inject(Gp, xpo, srow):
                    """Open the 4 per-unit-chunk accumulation groups with
                    the xproj slice for step row srow of ring tile xpo.
                    Row group 32*srow, col group 32*uc: all 4 concurrent,
                    and independent of the recurrent state."""
                    p0 = 32 * srow
                    for uc in range(4):
                        nc.tensor.matmul(
                            Gp[32 * uc : 32 * uc + 32, :],
                            lhsT=id_sb[p0 : p0 + 32, p0 : p0 + 32],
                            rhs=xpo[p0 : p0 + 32, 512 * uc : 512 * (uc + 1)],
                            start=True,
                            stop=False,
                            skip_group_check=True,
                            tile_position=(p0, 32 * uc),
                        )

                def umms(Gp):
                    """16 recurrent matmuls, round-robin across the 4 col
                    groups so the 4 unit-chunk chains stream concurrently."""
                    for k in range(4):
                        for uc in range(4):
                            nc.tensor.matmul(
                                Gp[32 * uc : 32 * uc + 32, :],
                                lhsT=hT_sb[:, 32 * k : 32 * k + 32],
                                rhs=u_sb[k][:, 512 * uc : 512 * (uc + 1)],
                                start=False,
                                stop=(k == 3),
                                skip_group_check=True,
                                tile_position=(0, 32 * uc),
                            )

                def chain(Gp):
                    """Activations + c/h update for one step. Everything
                    is a full-partition [128, *] tile at base 0. Acts are
                    ordered so the DVE chain starts as early as possible:
                    sigmoid(f,i) -> ct1/ct2 while tanh(chat) runs, then
                    sigmoid(o) off the critical path."""
                    gsb = gsbp.tile([128, 384], F32, tag="gsb")
                    cht = gsbp.tile([128, 128], F32, tag="cht")
                    nc.scalar.activation(gsb[:, 0:256], Gp[:, 0:256], AF.Sigmoid)
                    nc.scalar.activation(cht[:], Gp[:, 384:512], AF.Tanh)
                    ct1 = tmpp.tile([128, 128], F32, tag="ct1")
                    ct2 = tmpp.tile([128, 128], F32, tag="ct2")
                    thc = tmpp.tile([128, 128], F32, tag="thc")
                    nc.vector.tensor_mul(ct1[:], gsb[:, 0:128], c_sb[:])
                    nc.vector.tensor_mul(ct2[:], gsb[:, 128:256], cht[:])
                    nc.scalar.activation(gsb[:, 256:384], Gp[:, 256:384], AF.Sigmoid)
                    nc.vector.tensor_add(c_sb[:], ct1[:], ct2[:])
                    nc.scalar.activation(thc[:], c_sb[:], AF.Tanh)
                    hs = hsp.tile([128, 128], BF16, tag="hs")
                    nc.vector.tensor_mul(hs[:], gsb[:, 256:384], thc[:])
                    htp = psH.tile([128, 128], BF16, tag="htp")
                    nc.tensor.transpose(htp[:], hs[:], id_sb[:])
                    nc.vector.tensor_copy(hT_sb[:], htp[:])

                # ---- prologue: fill the ring for iteration 0 ----
                stage_tokens(0)
                for j in range(RING):
                    xpo = xpop.tile([128, G], BF16, tag="xpo")
                    producer(xpo, j)

                # ---- steady-state loop: 32 steps + 8 producers ----
                def iteration(iv):
                    stage_tokens(iv // 128 + RING)
                    xpo_objs = []
                    for _ in range(RING):
                        xpo = xpop.tile([128, G], BF16, tag="xpo")
                        xpo_objs.append(xpo)
                    Gcur = psG.tile([128, 512], F32, tag="G")
                    inject(Gcur, xpo_objs[0], 0)
                    for t in range(unroll):
                        j, s = divmod(t, 4)
                        umms(Gcur)
                        if t + 1 < unroll:
                            nxt = psG.tile([128, 512], F32, tag="G")
                            inject(nxt, xpo_objs[(t + 1) // 4], (t + 1) % 4)
                        else:
                            nxt = None
                        chain(Gcur)
                        if s == 3:
                            # produce this quad's ring slot for the next
                            # iteration; fills the chain's PE bubbles
                            producer(xpo_objs[j], j)
                        Gcur = nxt

                n_iters = T // unroll
                if n_iters == 1:
                    iteration(0)
                else:
                    with tc.For_i(
                        0,
                        rows,
                        B_LOC * unroll,
                        staggered_reset=True,
                        hint_engines=(
                            mybir.EngineType.PE,
                            mybir.EngineType.DVE,
                            mybir.EngineType.Activation,
                        ),
                    ) as iv:
                        iteration(iv)

                # final projection + sigmoid
                yps = psH.tile([B_LOC, 1], F32, tag="yps")
                for k in range(4):
                    nc.tensor.matmul(
                        yps[:],
                        lhsT=hT_sb[:, k * 32 : (k + 1) * 32],
                        rhs=wout_sb[:, k : k + 1],
                        start=(k == 0),
                        stop=(k == 3),
                    )
                ysb = tmpp.tile([B_LOC, 1], F32, tag="ysb")
                nc.scalar.activation(ysb[:], yps[:], AF.Sigmoid, bias=bout_sb[:, 0:1])
                nc.sync.dma_start(y_d[:], ysb[:])

    nc.compile()
    return nc


def prep_inputs(tokens, emb, Wf, Uf, bf, Wi, Ui, bi, Wc, Uc, bc, Wo, Uo, bo, W, b):
    """Host-side prep: concat gate weights gate-major [f|i|o|c], permute
    columns to unit-chunk-major order, cast to bf16, shard tokens."""
    bf16 = ml_dtypes.bfloat16
    perm = np.array(
        [
            512 * g + 128 * uc + u
            for uc in range(4)
            for g in range(4)
            for u in range(128)
        ]
    )
    wcat = np.concatenate([Wf, Wi, Wo, Wc], axis=1)[:, perm].astype(bf16)  # [E, G]
    ucat = np.concatenate([Uf, Ui, Uo, Uc], axis=1)[:, perm].astype(bf16)  # [U, G]
    bcat = np.concatenate([bf, bi, bo, bc], axis=0)[perm].astype(np.float32)  # [G]
    brow = bcat[None, :].astype(bf16)
    ones = np.ones((1, 128), bf16)
    emb_bf = np.asarray(emb, np.float32).astype(bf16)
    ident = np.eye(128, dtype=bf16)
    wout = np.ascontiguousarray(
        np.asarray(W, np.float32).reshape(4, 128).T
    ).astype(bf16)  # [128, 4]; wout[p, k] = W[k*128 + p]
    bout = np.full((B_LOC, 1), float(np.asarray(b).reshape(-1)[0]), np.float32)

    tokens = np.asarray(tokens)
    T = tokens.shape[1]
    n_mtiles = T * B_LOC // 128
    per_core = []
    for core in range(N_CORES):
        tok = tokens[core * B_LOC : (core + 1) * B_LOC]  # [B_LOC, T]
        tok_tm = np.ascontiguousarray(tok.T).reshape(-1)  # row = t*B_LOC + b
        tok_pm = np.ascontiguousarray(
            tok_tm.reshape(n_mtiles, 128).T
        ).astype(np.int32)  # [128, n_mtiles]
        # pad RING zero columns so the last iteration's lookahead
        # producers gather a valid (if unused) token
        tok_pm = np.concatenate(
            [tok_pm, np.zeros((128, RING), np.int32)], axis=1
        )
        per_core.append(
            dict(
                tokens_pm=tok_pm,
                emb=emb_bf,
                wcat=wcat,
                ucat=ucat,
                ones=ones,
                brow=brow,
                ident=ident,
                wout=wout,
                bout=bout,
            )
        )
    return per_core


_NC_CACHE = {}
LAST_RESULT = None


def kernel(**inputs):
    global LAST_RESULT
    key = "full"
    if key not in _NC_CACHE:
        _NC_CACHE[key] = build_nc()
    nc = _NC_CACHE[key]
    in_maps = prep_inputs(**inputs)
    res = run_bass_kernel_spmd(nc, in_maps, core_ids=list(range(N_CORES)))
    LAST_RESULT = res
    out = np.concatenate([r["y"] for r in res.results], axis=0)
    return out.astype(np.float32)
